# revision 1
# baseline (speedup 1.0000x reference)
"""GCN 2-layer encoder on 8 Trainium2 NeuronCores (Bass/Tile).

kernel(**inputs) takes the FULL inputs and returns the FULL [80000, 32] f32
output.  Strategy (node partition across 8 cores, per sharding hint):

  gcn_conv(x, W, b) = b + dinv * (A_hat @ (dinv * (x @ W)))  with self-loops,
  where dinv = 1/sqrt(indeg+1) and A_hat is the (unnormalized) adjacency.

  Launch A: z1 = dinv * (x @ W1)        (each core transforms its node shard)
  Launch B: per dst-node-tile (128 nodes) gather z1 rows by edge source
            (gpsimd dma_gather, 256B rows) and reduce with a one-hot
            scatter-matmul on the PE into PSUM; epilogue
            z2 = relu(dinv*agg + b1) * dinv.
  Launch C: same aggregation over z2; epilogue out = (dinv*agg) @ W2 + b2.

  Host relays the (small) per-shard results between launches, because each
  layer's gather needs all cores' rows.  Edges are grouped by
  (dst tile, src range) with counts padded to a global per-rank max so all
  8 cores run one identical SPMD program; int16 gather indices are relative
  to one of three <=32768-row source ranges.
"""
import sys
import time

sys.path.insert(0, '/opt/trn_rl_repo')

import numpy as np
import jax
from jax.sharding import Mesh, PartitionSpec
from jax.experimental.shard_map import shard_map

import concourse.bass as bass
import concourse.bacc as bacc
import concourse.tile as tile
import concourse.mybir as mybir
from concourse import bass2jax
from concourse.bass2jax import _bass_exec_p, partition_id_tensor
from concourse.masks import make_identity

F32 = mybir.dt.float32
I16 = mybir.dt.int16

N_NODES = 80000
IN_CH = 128
HID = 64
OUT_CH = 32
N_CORES = 8
RANGE_CAP = 32768
CALL_MAX = 896
NT = N_NODES // 128                     # 625 dst tiles
TPC = (NT + N_CORES - 1) // N_CORES     # 79 tile ranks per core
R_BASES = np.array([0, 32768, 65536], np.int64)
R_SIZES = np.array([32768, 32768, 14464], np.int64)
R = 3


def _ceil128(x):
    return ((x + 127) // 128) * 128


def _preprocess(edge_index):
    src = np.asarray(edge_index[0], np.int64)
    dst = np.asarray(edge_index[1], np.int64)
    deg = np.bincount(dst, minlength=N_NODES).astype(np.float64) + 1.0
    dinv = (1.0 / np.sqrt(deg)).astype(np.float32)
    loop = np.arange(N_NODES, dtype=np.int64)
    s_all = np.concatenate([src, loop])
    d_all = np.concatenate([dst, loop])
    tile_g = d_all >> 7
    r_id = np.searchsorted(R_BASES[1:], s_all, side='right')

    core_of_tile = np.minimum(np.arange(NT) // TPC, N_CORES - 1)
    cnt_tile_r = np.bincount(tile_g * R + r_id, minlength=NT * R).reshape(NT, R)
    tot_tile = cnt_tile_r.sum(1)

    tile_of = -np.ones((N_CORES, TPC), np.int64)
    cntK = np.zeros((N_CORES, TPC, R), np.int64)
    for c in range(N_CORES):
        tl = np.where(core_of_tile == c)[0]
        order = tl[np.argsort(-tot_tile[tl], kind='stable')]
        tile_of[c, :len(order)] = order
        cntK[c, :len(order)] = cnt_tile_r[order]
    K = _ceil128(cntK.max(axis=0))
    Koff = np.zeros(TPC * R + 1, np.int64)
    Koff[1:] = np.cumsum(K.reshape(-1))
    T_pad = int(Koff[-1])
    C_total = T_pad // 128

    gidx_all = np.zeros((N_CORES, T_pad), np.int16)
    dstl_all = np.full((N_CORES, T_pad), -1.0, np.float32)
    ecore = core_of_tile[tile_g]
    for c in range(N_CORES):
        rank_of = np.full(NT, -1, np.int64)
        real = tile_of[c][tile_of[c] >= 0]
        rank_of[real] = np.arange(len(real))
        m = ecore == c
        e_rank = rank_of[tile_g[m]]
        e_r = r_id[m]
        seg = e_rank * R + e_r
        order = np.argsort(seg, kind='stable')
        seg_s = seg[order]
        seg_counts = np.bincount(seg_s, minlength=TPC * R)
        starts = np.zeros(TPC * R, np.int64)
        starts[1:] = np.cumsum(seg_counts)[:-1]
        within = np.arange(len(seg_s)) - starts[seg_s]
        pos = Koff[seg_s] + within
        gidx_all[c, pos] = (s_all[m][order] - R_BASES[e_r[order]]).astype(np.int16)
        dstl_all[c, pos] = (d_all[m][order] & 127).astype(np.float32)

    calls = []
    for t in range(TPC):
        for r in range(R):
            k = int(K[t, r])
            off = int(Koff[t * R + r])
            while k > 0:
                sz = min(k, CALL_MAX)
                calls.append((t, r, off, sz))
                off += sz
                k -= sz

    idxw_all = np.zeros((N_CORES, 128, T_pad // 16), np.int16)
    for c in range(N_CORES):
        for (_, _, off, sz) in calls:
            blk = gidx_all[c, off:off + sz].reshape(sz // 16, 16).T
            idxw_all[c, :, off // 16:(off + sz) // 16] = np.tile(blk, (8, 1))
    dstv_all = dstl_all.reshape(N_CORES, C_total, 128).transpose(0, 2, 1).copy()

    dinvS = np.ones((N_CORES, 128, TPC), np.float32)
    for c in range(N_CORES):
        for t in range(TPC):
            tl = tile_of[c, t]
            if tl >= 0:
                dinvS[c, :, t] = dinv[tl * 128:(tl + 1) * 128]

    return dict(dinv=dinv, tile_of=tile_of, K=K, T_pad=T_pad,
                C_total=C_total, calls=calls, idxw=idxw_all, dstv=dstv_all,
                dinvS=dinvS)


def _build_A():
    nc = bacc.Bacc("TRN2", target_bir_lowering=False, debug=False,
                   num_devices=N_CORES, num_swdge_queues=4)
    rows = TPC * 128
    x_d = nc.dram_tensor("x", [rows, IN_CH], F32, kind="ExternalInput")
    w1_d = nc.dram_tensor("w1", [IN_CH, HID], F32, kind="ExternalInput")
    dinv_d = nc.dram_tensor("dinv", [128, TPC], F32, kind="ExternalInput")
    z_d = nc.dram_tensor("z", [rows, HID], F32, kind="ExternalOutput")
    with tile.TileContext(nc) as tc:
        with (
            tc.tile_pool(name="const", bufs=1) as cp,
            tc.tile_pool(name="xin", bufs=4) as xp,
            tc.tile_pool(name="xt", bufs=4) as xtp,
            tc.tile_pool(name="zs", bufs=4) as zp,
            tc.tile_pool(name="ps", bufs=4, space="PSUM") as ps,
            tc.tile_pool(name="pst", bufs=4, space="PSUM") as pst,
        ):
            ident = cp.tile([128, 128], F32)
            make_identity(nc, ident[:])
            w1sb = cp.tile([IN_CH, HID], F32)
            nc.sync.dma_start(out=w1sb[:], in_=w1_d.ap()[:, :])
            dinv_sb = cp.tile([128, TPC], F32)
            nc.sync.dma_start(out=dinv_sb[:], in_=dinv_d.ap()[:, :])
            for t in range(TPC):
                xt_in = xp.tile([128, IN_CH], F32)
                nc.sync.dma_start(out=xt_in[:],
                                  in_=x_d.ap()[t * 128:(t + 1) * 128, :])
                xT_ps = pst.tile([IN_CH, 128], F32, space="PSUM")
                nc.tensor.transpose(out=xT_ps[:], in_=xt_in[:], identity=ident[:])
                xT = xtp.tile([IN_CH, 128], F32)
                nc.vector.tensor_copy(out=xT[:], in_=xT_ps[:])
                zps = ps.tile([128, HID], F32, space="PSUM")
                nc.tensor.matmul(out=zps[:], lhsT=xT[:], rhs=w1sb[:],
                                 start=True, stop=True)
                zsb = zp.tile([128, HID], F32)
                nc.vector.tensor_scalar(out=zsb[:], in0=zps[:],
                                        scalar1=dinv_sb[:, t:t + 1], scalar2=None,
                                        op0=mybir.AluOpType.mult)
                nc.sync.dma_start(out=z_d.ap()[t * 128:(t + 1) * 128, :],
                                  in_=zsb[:])
    nc.compile()
    return nc


def _build_agg(pre, layer):
    K, calls, C_total, T_pad = pre["K"], pre["calls"], pre["C_total"], pre["T_pad"]
    OC = HID if layer == 1 else OUT_CH
    nc = bacc.Bacc("TRN2", target_bir_lowering=False, debug=False,
                   num_devices=N_CORES, num_swdge_queues=4)
    zin_d = nc.dram_tensor("zin", [N_NODES, HID], F32, kind="ExternalInput")
    idx_d = nc.dram_tensor("idxw", [128, T_pad // 16], I16, kind="ExternalInput")
    dstv_d = nc.dram_tensor("dstv", [128, C_total], F32, kind="ExternalInput")
    dinv_d = nc.dram_tensor("dinvS", [128, TPC], F32, kind="ExternalInput")
    bb_d = nc.dram_tensor("bb", [128, OC], F32, kind="ExternalInput")
    if layer == 2:
        w2_d = nc.dram_tensor("w2", [HID, OUT_CH], F32, kind="ExternalInput")
    out_d = nc.dram_tensor("outp", [TPC * 128, OC], F32, kind="ExternalOutput")

    calls_of = [[] for _ in range(TPC)]
    for (t, r, off, sz) in calls:
        calls_of[t].append((r, off, sz))

    with tile.TileContext(nc) as tc:
        with (
            tc.tile_pool(name="const", bufs=1) as cp,
            tc.tile_pool(name="msgs", bufs=8) as mp,
            tc.tile_pool(name="s4", bufs=4) as sp,
            tc.tile_pool(name="ep", bufs=4) as ep,
            tc.tile_pool(name="ps", bufs=4 if layer == 1 else 3,
                         space="PSUM") as ps,
            tc.tile_pool(name="ps2", bufs=2, space="PSUM") as ps2,
        ):
            idx_sb = cp.tile([128, T_pad // 16], I16)
            nc.sync.dma_start(out=idx_sb[:], in_=idx_d.ap()[:, :])
            dstv_sb = cp.tile([128, C_total], F32)
            nc.sync.dma_start(out=dstv_sb[:], in_=dstv_d.ap()[:, :])
            dinv_sb = cp.tile([128, TPC], F32)
            nc.sync.dma_start(out=dinv_sb[:], in_=dinv_d.ap()[:, :])
            bb_sb = cp.tile([128, OC], F32)
            nc.sync.dma_start(out=bb_sb[:], in_=bb_d.ap()[:, :])
            iota_i = cp.tile([128, 512], I16)
            nc.gpsimd.iota(iota_i[:], pattern=[[0, 4], [1, 128]], base=0,
                           channel_multiplier=0)
            iota4 = cp.tile([128, 4, 128], F32)
            nc.vector.tensor_copy(out=iota4[:],
                                  in_=iota_i[:].rearrange("p (c f) -> p c f", c=4))
            if layer == 2:
                ident = cp.tile([128, 128], F32)
                make_identity(nc, ident[:])
                w2sb = cp.tile([HID, OUT_CH], F32)
                nc.sync.dma_start(out=w2sb[:], in_=w2_d.ap()[:, :])

            qn = 0
            chunk_g = 0
            S4 = None
            for t in range(TPC):
                nchunks_t = int(K[t].sum()) // 128
                if nchunks_t == 0:
                    continue
                psum_t = ps.tile([128, HID], F32, space="PSUM")
                mtiles = []
                for (r, off, sz) in calls_of[t]:
                    m = mp.tile([128, CALL_MAX // 128, HID], F32, tag="msgs")
                    base = int(R_BASES[r])
                    size_r = int(R_SIZES[r])
                    nc.gpsimd.dma_gather(
                        out_ap=m[:, :sz // 128, :],
                        in_ap=zin_d.ap()[base:base + size_r, :],
                        idxs_ap=idx_sb[:, off // 16:(off + sz) // 16],
                        num_idxs=sz,
                        num_idxs_reg=sz,
                        elem_size=HID,
                        single_packet=True,
                        queue_num=qn % 4,
                    )
                    qn += 1
                    mtiles.append((m, sz // 128))
                ci = 0
                for (m, nslots) in mtiles:
                    for s in range(nslots):
                        if chunk_g % 4 == 0:
                            g0 = chunk_g
                            gw = min(4, C_total - g0)
                            S4 = sp.tile([128, 4, 128], F32, tag="s4")
                            dv = dstv_sb[:, g0:g0 + gw, None]\
                                .to_broadcast([128, gw, 128])
                            nc.vector.tensor_tensor(
                                out=S4[:, :gw, :], in0=iota4[:, :gw, :],
                                in1=dv, op=mybir.AluOpType.is_equal)
                        nc.tensor.matmul(out=psum_t[:, :],
                                         lhsT=S4[:, chunk_g % 4, :],
                                         rhs=m[:, s, :],
                                         start=(ci == 0),
                                         stop=(ci == nchunks_t - 1))
                        ci += 1
                        chunk_g += 1
                if layer == 1:
                    t1 = ep.tile([128, HID], F32, tag="t1")
                    nc.vector.tensor_scalar(out=t1[:], in0=psum_t[:],
                                            scalar1=dinv_sb[:, t:t + 1],
                                            scalar2=None,
                                            op0=mybir.AluOpType.mult)
                    t2 = ep.tile([128, HID], F32, tag="t2")
                    nc.vector.tensor_tensor(out=t2[:], in0=t1[:], in1=bb_sb[:],
                                            op=mybir.AluOpType.add)
                    z2 = ep.tile([128, HID], F32, tag="z2")
                    nc.scalar.activation(out=z2[:], in_=t2[:],
                                         func=mybir.ActivationFunctionType.Relu,
                                         scale=dinv_sb[:, t:t + 1])
                    nc.sync.dma_start(out=out_d.ap()[t * 128:(t + 1) * 128, :],
                                      in_=z2[:])
                else:
                    t1 = ep.tile([128, HID], F32, tag="t1")
                    nc.vector.tensor_scalar(out=t1[:], in0=psum_t[:],
                                            scalar1=dinv_sb[:, t:t + 1],
                                            scalar2=None,
                                            op0=mybir.AluOpType.mult)
                    tT_ps = ps2.tile([HID, 128], F32, space="PSUM", tag="tT")
                    nc.tensor.transpose(out=tT_ps[:], in_=t1[:],
                                        identity=ident[:])
                    tT = ep.tile([HID, 128], F32, tag="tT_sb")
                    nc.vector.tensor_copy(out=tT[:], in_=tT_ps[:])
                    ops = ps2.tile([128, OUT_CH], F32, space="PSUM", tag="o")
                    nc.tensor.matmul(out=ops[:], lhsT=tT[:], rhs=w2sb[:],
                                     start=True, stop=True)
                    o = ep.tile([128, OUT_CH], F32, tag="o_sb")
                    nc.vector.tensor_tensor(out=o[:], in0=ops[:], in1=bb_sb[:],
                                            op=mybir.AluOpType.add)
                    nc.sync.dma_start(out=out_d.ap()[t * 128:(t + 1) * 128, :],
                                      in_=o[:])
    nc.compile()
    return nc


class _SpmdRunner:
    def __init__(self, nc, n_cores=N_CORES):
        bass2jax.install_neuronx_cc_hook()
        self.nc = nc
        self.n_cores = n_cores
        in_names, out_names, out_avals = [], [], []
        partition_name = nc.partition_id_tensor.name if nc.partition_id_tensor \
            else None
        for alloc in nc.m.functions[0].allocations:
            if not isinstance(alloc, mybir.MemoryLocationSet):
                continue
            name = alloc.memorylocations[0].name
            if alloc.kind == "ExternalInput":
                if name != partition_name:
                    in_names.append(name)
            elif alloc.kind == "ExternalOutput":
                out_names.append(name)
                out_avals.append(jax.core.ShapedArray(
                    tuple(alloc.tensor_shape), mybir.dt.np(alloc.dtype)))
        self.in_names, self.out_names, self.out_avals = \
            in_names, out_names, out_avals
        n_params = len(in_names)
        n_outs = len(out_avals)
        all_names = list(in_names) + list(out_names)
        if partition_name is not None:
            all_names.append(partition_name)

        def _body(*args):
            operands = list(args)
            if partition_name is not None:
                operands.append(partition_id_tensor())
            outs = _bass_exec_p.bind(
                *operands,
                out_avals=tuple(out_avals),
                in_names=tuple(all_names),
                out_names=tuple(out_names),
                lowering_input_output_aliases=(),
                sim_require_finite=True,
                sim_require_nnan=True,
                nc=nc,
            )
            return tuple(outs)

        devices = jax.devices()[:n_cores]
        assert len(devices) >= n_cores or len(devices) == n_cores, \
            f"need {n_cores} cores, have {len(jax.devices())}"
        self.mesh = Mesh(np.asarray(devices), ("core",))
        in_specs = (PartitionSpec("core"),) * (n_params + n_outs)
        out_specs = (PartitionSpec("core"),) * n_outs
        self.fn = jax.jit(
            shard_map(_body, mesh=self.mesh, in_specs=in_specs,
                      out_specs=out_specs, check_rep=False),
            keep_unused=True,
        )

    def run(self, in_maps):
        concat_in = [
            np.concatenate([np.asarray(in_maps[c][nm])
                            for c in range(self.n_cores)], axis=0)
            for nm in self.in_names
        ]
        concat_zeros = [
            np.zeros((self.n_cores * av.shape[0], *av.shape[1:]), av.dtype)
            for av in self.out_avals
        ]
        outs = self.fn(*(concat_in + concat_zeros))
        jax.block_until_ready(outs)
        res = []
        for c in range(self.n_cores):
            d = {}
            for i, nm in enumerate(self.out_names):
                a = np.asarray(outs[i]).reshape(self.n_cores,
                                                *self.out_avals[i].shape)
                d[nm] = a[c]
            res.append(d)
        return res


_CACHE = {}


def _get_programs(edge_index):
    key = hash(np.asarray(edge_index).tobytes())
    if key not in _CACHE:
        pre = _preprocess(edge_index)
        ncA = _build_A()
        ncB = _build_agg(pre, layer=1)
        ncC = _build_agg(pre, layer=2)
        _CACHE[key] = (pre, _SpmdRunner(ncA), _SpmdRunner(ncB),
                       _SpmdRunner(ncC))
    return _CACHE[key]


def kernel(x, edge_index, W1, b1, W2, b2):
    x = np.asarray(x, np.float32)
    W1 = np.asarray(W1, np.float32)
    b1 = np.asarray(b1, np.float32)
    W2 = np.asarray(W2, np.float32)
    b2 = np.asarray(b2, np.float32)
    pre, rA, rB, rC = _get_programs(edge_index)
    dinv, tile_of = pre["dinv"], pre["tile_of"]

    # ---- launch A: z1 = dinv * (x @ W1), natural node order shards
    rows = TPC * 128
    mapsA = []
    for c in range(N_CORES):
        lo = c * TPC * 128
        hi = min((c + 1) * TPC * 128, N_NODES)
        xs = np.zeros((rows, IN_CH), np.float32)
        xs[:hi - lo] = x[lo:hi]
        dv = np.ones((128, TPC), np.float32)
        dv[:, :(hi - lo) // 128] = dinv[lo:hi].reshape(-1, 128).T
        mapsA.append({"x": xs, "w1": W1, "dinv": dv})
    resA = rA.run(mapsA)
    z1 = np.zeros((N_NODES, HID), np.float32)
    for c in range(N_CORES):
        lo = c * TPC * 128
        hi = min((c + 1) * TPC * 128, N_NODES)
        z1[lo:hi] = resA[c]["z"][:hi - lo]

    # ---- launch B: layer-1 aggregation -> z2 (staged rank order per core)
    bb1 = np.tile(b1, (128, 1)).astype(np.float32)
    mapsB = [{"zin": z1, "idxw": pre["idxw"][c], "dstv": pre["dstv"][c],
              "dinvS": pre["dinvS"][c], "bb": bb1} for c in range(N_CORES)]
    resB = rB.run(mapsB)
    z2 = np.zeros((N_NODES, HID), np.float32)
    for c in range(N_CORES):
        o = resB[c]["outp"]
        for t in range(TPC):
            tl = tile_of[c, t]
            if tl >= 0:
                z2[tl * 128:(tl + 1) * 128] = o[t * 128:(t + 1) * 128]

    # ---- launch C: layer-2 aggregation -> final output
    bb2 = np.tile(b2, (128, 1)).astype(np.float32)
    mapsC = [{"zin": z2, "idxw": pre["idxw"][c], "dstv": pre["dstv"][c],
              "dinvS": pre["dinvS"][c], "bb": bb2, "w2": W2}
             for c in range(N_CORES)]
    resC = rC.run(mapsC)
    out = np.zeros((N_NODES, OUT_CH), np.float32)
    for c in range(N_CORES):
        o = resC[c]["outp"]
        for t in range(TPC):
            tl = tile_of[c, t]
            if tl >= 0:
                out[tl * 128:(tl + 1) * 128] = o[t * 128:(t + 1) * 128]
    return out



# revision 2
# speedup vs baseline: 3.9069x; 3.9069x over previous
"""GCN 2-layer encoder on 8 Trainium2 NeuronCores (Bass/Tile).

kernel(**inputs) takes the FULL inputs and returns the FULL [80000, 32] f32
output.  Strategy (node partition across 8 cores, per sharding hint), ONE
fused SPMD launch with two in-kernel AllGathers:

  gcn_conv(x, W, b) = b + dinv * (A_hat @ (dinv * (x @ W)))  with self-loops,
  where dinv = 1/sqrt(indeg+1) and A_hat is the (unnormalized) adjacency.

  Phase A: z1 = dinv * (x @ W1)         (each core, its 10112-row node shard)
  AllGather(z1) -> every core holds all 80896 rows (rank-major blocks).
  Phase B: per dst-node-tile (128 nodes) gather z1 rows by edge source
           (gpsimd dma_gather, 256B rows) and reduce with a one-hot
           scatter-matmul on the PE into PSUM; epilogue
           z2 = relu(dinv*agg + b1) * dinv  (written rank-ordered).
  AllGather(z2) -> full z2 (rank-major layout; gather indices for layer 2
           are precomputed against this permuted layout).
  Phase C: same aggregation over z2; epilogue out = (dinv*agg) @ W2 + b2.

  Edges are grouped by (dst tile rank, src range) with counts padded to a
  global per-rank max so all 8 cores run one identical SPMD program; int16
  gather indices are relative to one of three <=32768-row source ranges,
  and are sorted by source address within each group for HBM locality.
"""
import sys

sys.path.insert(0, '/opt/trn_rl_repo')

import numpy as np
import jax
from jax.sharding import Mesh, PartitionSpec
from jax.experimental.shard_map import shard_map

import concourse.bass as bass
import concourse.bacc as bacc
import concourse.tile as tile
import concourse.mybir as mybir
from concourse import bass2jax
from concourse.bass2jax import _bass_exec_p, partition_id_tensor
from concourse.masks import make_identity

F32 = mybir.dt.float32
I16 = mybir.dt.int16

N_NODES = 80000
IN_CH = 128
HID = 64
OUT_CH = 32
N_CORES = 8
CALL_MAX = 896
NT = N_NODES // 128                     # 625 dst tiles
TPC = (NT + N_CORES - 1) // N_CORES     # 79 tile ranks per core
ROWS = TPC * 128                        # 10112 rows per core
TOT = N_CORES * ROWS                    # 80896 gathered rows
R_BASES = np.array([0, 32768, 65536], np.int64)
R_SIZES = np.array([32768, 32768, TOT - 65536], np.int64)
R = 3


def _ceil128(x):
    return ((x + 127) // 128) * 128


def _layer_pre(s_pos, d_all, core_of_tile, tile_of, rank_of_tile):
    """Per-layer gather/scatter metadata given source *positions* s_pos
    (natural node ids for layer 1; rank-major permuted positions for
    layer 2) and destinations d_all."""
    tile_g = d_all >> 7
    r_id = np.searchsorted(R_BASES[1:], s_pos, side='right')
    cnt_tile_r = np.bincount(tile_g * R + r_id,
                             minlength=NT * R).reshape(NT, R)
    cntK = np.zeros((N_CORES, TPC, R), np.int64)
    for c in range(N_CORES):
        real = tile_of[c][tile_of[c] >= 0]
        cntK[c, :len(real)] = cnt_tile_r[real]
    K = _ceil128(cntK.max(axis=0))
    Koff = np.zeros(TPC * R + 1, np.int64)
    Koff[1:] = np.cumsum(K.reshape(-1))
    T_pad = int(Koff[-1])
    C_total = T_pad // 128

    gidx_all = np.zeros((N_CORES, T_pad), np.int16)
    dstl_all = np.full((N_CORES, T_pad), -1.0, np.float32)
    ecore = core_of_tile[tile_g]
    for c in range(N_CORES):
        m = ecore == c
        e_rank = rank_of_tile[tile_g[m]]
        e_r = r_id[m]
        seg = e_rank * R + e_r
        sp = s_pos[m]
        order = np.lexsort((sp, seg))       # by segment, then src address
        seg_s = seg[order]
        seg_counts = np.bincount(seg_s, minlength=TPC * R)
        starts = np.zeros(TPC * R, np.int64)
        starts[1:] = np.cumsum(seg_counts)[:-1]
        within = np.arange(len(seg_s)) - starts[seg_s]
        pos = Koff[seg_s] + within
        gidx_all[c, pos] = (sp[order] - R_BASES[e_r[order]]).astype(np.int16)
        dstl_all[c, pos] = (d_all[m][order] & 127).astype(np.float32)

    calls = []
    for t in range(TPC):
        for r in range(R):
            k = int(K[t, r])
            off = int(Koff[t * R + r])
            while k > 0:
                sz = min(k, CALL_MAX)
                calls.append((t, r, off, sz))
                off += sz
                k -= sz

    idxw_all = np.zeros((N_CORES, 128, T_pad // 16), np.int16)
    for c in range(N_CORES):
        blk = gidx_all[c].reshape(T_pad // 16, 16).T
        idxw_all[c] = np.tile(blk, (8, 1))
    dstv_all = dstl_all.reshape(N_CORES, C_total, 128).transpose(0, 2, 1).copy()

    return dict(K=K, calls=calls, T_pad=T_pad, C_total=C_total,
                idxw=idxw_all, dstv=dstv_all)


def _preprocess(edge_index):
    src = np.asarray(edge_index[0], np.int64)
    dst = np.asarray(edge_index[1], np.int64)
    deg = np.bincount(dst, minlength=N_NODES).astype(np.float64) + 1.0
    dinv = (1.0 / np.sqrt(deg)).astype(np.float32)
    loop = np.arange(N_NODES, dtype=np.int64)
    s_all = np.concatenate([src, loop])
    d_all = np.concatenate([dst, loop])
    tile_g = d_all >> 7

    core_of_tile = np.minimum(np.arange(NT) // TPC, N_CORES - 1)
    tot_tile = np.bincount(tile_g, minlength=NT)
    tile_of = -np.ones((N_CORES, TPC), np.int64)
    for c in range(N_CORES):
        tl = np.where(core_of_tile == c)[0]
        order = tl[np.argsort(-tot_tile[tl], kind='stable')]
        tile_of[c, :len(order)] = order
    rank_of_tile = np.zeros(NT, np.int64)
    for c in range(N_CORES):
        real = tile_of[c][tile_of[c] >= 0]
        rank_of_tile[real] = np.arange(len(real))

    # rank-major position of node n inside the AllGather'd z2 buffer
    t_of_n = np.arange(N_NODES) >> 7
    perm = ((core_of_tile[t_of_n] * TPC + rank_of_tile[t_of_n]) * 128
            + (np.arange(N_NODES) & 127))

    L1 = _layer_pre(s_all, d_all, core_of_tile, tile_of, rank_of_tile)
    L2 = _layer_pre(perm[s_all], d_all, core_of_tile, tile_of, rank_of_tile)

    dinvS = np.ones((N_CORES, 128, TPC), np.float32)
    for c in range(N_CORES):
        for t in range(TPC):
            tl = tile_of[c, t]
            if tl >= 0:
                dinvS[c, :, t] = dinv[tl * 128:(tl + 1) * 128]
    dinvA = np.ones((N_CORES, 128, TPC), np.float32)
    for c in range(N_CORES):
        lo = c * ROWS
        hi = min((c + 1) * ROWS, N_NODES)
        dinvA[c, :, :(hi - lo) // 128] = dinv[lo:hi].reshape(-1, 128).T

    return dict(dinv=dinv, tile_of=tile_of, L1=L1, L2=L2,
                dinvS=dinvS, dinvA=dinvA)


def _emit_agg(nc, tc, cp, src_ap, idx_sb, dstv_sb, dinv_sb, bb_sb, iota4,
              L, layer, out_d, w2sb=None, ident=None):
    """Emit one aggregation phase: gather rows of src_ap per edge, scatter-
    matmul per dst tile into PSUM, apply the layer epilogue, write to out_d
    (a DRAM AP indexable by [row0:row1, :])."""
    K, calls, C_total = L["K"], L["calls"], L["C_total"]
    calls_of = [[] for _ in range(TPC)]
    for (t, r, off, sz) in calls:
        calls_of[t].append((r, off, sz))
    tg = f"l{layer}"

    with (
        tc.tile_pool(name=f"msgs{layer}", bufs=8) as mp,
        tc.tile_pool(name=f"s4_{layer}", bufs=4) as sp,
        tc.tile_pool(name=f"ep{layer}", bufs=4) as ep,
        tc.tile_pool(name=f"ps{layer}", bufs=4 if layer == 1 else 3,
                     space="PSUM") as ps,
        tc.tile_pool(name=f"ps2_{layer}", bufs=2, space="PSUM") as ps2,
    ):
        qn = 0
        chunk_g = 0
        S4 = None
        for t in range(TPC):
            nchunks_t = int(K[t].sum()) // 128
            if nchunks_t == 0:
                continue
            psum_t = ps.tile([128, HID], F32, space="PSUM")
            mtiles = []
            for (r, off, sz) in calls_of[t]:
                m = mp.tile([128, CALL_MAX // 128, HID], F32, tag=f"msgs{tg}")
                base = int(R_BASES[r])
                size_r = int(R_SIZES[r])
                nc.gpsimd.dma_gather(
                    out_ap=m[:, :sz // 128, :],
                    in_ap=src_ap[base:base + size_r, :],
                    idxs_ap=idx_sb[:, off // 16:(off + sz) // 16],
                    num_idxs=sz,
                    num_idxs_reg=sz,
                    elem_size=HID,
                    single_packet=True,
                    queue_num=qn % 4,
                )
                qn += 1
                mtiles.append((m, sz // 128))
            ci = 0
            for (m, nslots) in mtiles:
                for s in range(nslots):
                    if chunk_g % 4 == 0:
                        g0 = chunk_g
                        gw = min(4, C_total - g0)
                        S4 = sp.tile([128, 4, 128], F32, tag=f"s4{tg}")
                        dv = dstv_sb[:, g0:g0 + gw, None]\
                            .to_broadcast([128, gw, 128])
                        nc.vector.tensor_tensor(
                            out=S4[:, :gw, :], in0=iota4[:, :gw, :],
                            in1=dv, op=mybir.AluOpType.is_equal)
                    nc.tensor.matmul(out=psum_t[:, :],
                                     lhsT=S4[:, chunk_g % 4, :],
                                     rhs=m[:, s, :],
                                     start=(ci == 0),
                                     stop=(ci == nchunks_t - 1))
                    ci += 1
                    chunk_g += 1
            if layer == 1:
                t1 = ep.tile([128, HID], F32, tag=f"t1{tg}")
                nc.vector.tensor_scalar(out=t1[:], in0=psum_t[:],
                                        scalar1=dinv_sb[:, t:t + 1],
                                        scalar2=None,
                                        op0=mybir.AluOpType.mult)
                t2 = ep.tile([128, HID], F32, tag=f"t2{tg}")
                nc.vector.tensor_tensor(out=t2[:], in0=t1[:], in1=bb_sb[:],
                                        op=mybir.AluOpType.add)
                z2 = ep.tile([128, HID], F32, tag=f"z2{tg}")
                nc.scalar.activation(out=z2[:], in_=t2[:],
                                     func=mybir.ActivationFunctionType.Relu,
                                     scale=dinv_sb[:, t:t + 1])
                nc.sync.dma_start(out=out_d[t * 128:(t + 1) * 128, :],
                                  in_=z2[:])
            else:
                t1 = ep.tile([128, HID], F32, tag=f"t1{tg}")
                nc.vector.tensor_scalar(out=t1[:], in0=psum_t[:],
                                        scalar1=dinv_sb[:, t:t + 1],
                                        scalar2=None,
                                        op0=mybir.AluOpType.mult)
                tT_ps = ps2.tile([HID, 128], F32, space="PSUM", tag=f"tT{tg}")
                nc.tensor.transpose(out=tT_ps[:], in_=t1[:],
                                    identity=ident[:])
                tT = ep.tile([HID, 128], F32, tag=f"tT_sb{tg}")
                nc.vector.tensor_copy(out=tT[:], in_=tT_ps[:])
                ops = ps2.tile([128, OUT_CH], F32, space="PSUM", tag=f"o{tg}")
                nc.tensor.matmul(out=ops[:], lhsT=tT[:], rhs=w2sb[:],
                                 start=True, stop=True)
                o = ep.tile([128, OUT_CH], F32, tag=f"o_sb{tg}")
                nc.vector.tensor_tensor(out=o[:], in0=ops[:], in1=bb_sb[:],
                                        op=mybir.AluOpType.add)
                nc.sync.dma_start(out=out_d[t * 128:(t + 1) * 128, :],
                                  in_=o[:])


def _build_merged(pre):
    L1, L2 = pre["L1"], pre["L2"]
    nc = bacc.Bacc("TRN2", target_bir_lowering=False, debug=False,
                   num_devices=N_CORES, num_swdge_queues=4)
    x_d = nc.dram_tensor("x", [ROWS, IN_CH], F32, kind="ExternalInput")
    w1_d = nc.dram_tensor("w1", [IN_CH, HID], F32, kind="ExternalInput")
    w2_d = nc.dram_tensor("w2", [HID, OUT_CH], F32, kind="ExternalInput")
    dinvA_d = nc.dram_tensor("dinvA", [128, TPC], F32, kind="ExternalInput")
    dinvS_d = nc.dram_tensor("dinvS", [128, TPC], F32, kind="ExternalInput")
    bb1_d = nc.dram_tensor("bb1", [128, HID], F32, kind="ExternalInput")
    bb2_d = nc.dram_tensor("bb2", [128, OUT_CH], F32, kind="ExternalInput")
    idx1_d = nc.dram_tensor("idxw1", [128, L1["T_pad"] // 16], I16,
                            kind="ExternalInput")
    dstv1_d = nc.dram_tensor("dstv1", [128, L1["C_total"]], F32,
                             kind="ExternalInput")
    idx2_d = nc.dram_tensor("idxw2", [128, L2["T_pad"] // 16], I16,
                            kind="ExternalInput")
    dstv2_d = nc.dram_tensor("dstv2", [128, L2["C_total"]], F32,
                             kind="ExternalInput")
    out_d = nc.dram_tensor("outp", [ROWS, OUT_CH], F32, kind="ExternalOutput")

    with tile.TileContext(nc) as tc:
        with (
            tc.tile_pool(name="const", bufs=1) as cp,
            tc.tile_pool(name="dram", bufs=1, space="DRAM") as dram,
        ):
            ag1_in = dram.tile([ROWS, HID], F32)
            ag1_out = dram.tile([TOT, HID], F32, addr_space="Shared")
            ag2_in = dram.tile([ROWS, HID], F32)
            ag2_out = dram.tile([TOT, HID], F32, addr_space="Shared")

            ident = cp.tile([128, 128], F32)
            make_identity(nc, ident[:])
            w1sb = cp.tile([IN_CH, HID], F32)
            nc.sync.dma_start(out=w1sb[:], in_=w1_d.ap()[:, :])
            w2sb = cp.tile([HID, OUT_CH], F32)
            nc.sync.dma_start(out=w2sb[:], in_=w2_d.ap()[:, :])
            dinvA_sb = cp.tile([128, TPC], F32)
            nc.sync.dma_start(out=dinvA_sb[:], in_=dinvA_d.ap()[:, :])
            dinvS_sb = cp.tile([128, TPC], F32)
            nc.sync.dma_start(out=dinvS_sb[:], in_=dinvS_d.ap()[:, :])
            bb1_sb = cp.tile([128, HID], F32)
            nc.sync.dma_start(out=bb1_sb[:], in_=bb1_d.ap()[:, :])
            bb2_sb = cp.tile([128, OUT_CH], F32)
            nc.sync.dma_start(out=bb2_sb[:], in_=bb2_d.ap()[:, :])
            idx1_sb = cp.tile([128, L1["T_pad"] // 16], I16)
            nc.sync.dma_start(out=idx1_sb[:], in_=idx1_d.ap()[:, :])
            dstv1_sb = cp.tile([128, L1["C_total"]], F32)
            nc.sync.dma_start(out=dstv1_sb[:], in_=dstv1_d.ap()[:, :])
            idx2_sb = cp.tile([128, L2["T_pad"] // 16], I16)
            nc.sync.dma_start(out=idx2_sb[:], in_=idx2_d.ap()[:, :])
            dstv2_sb = cp.tile([128, L2["C_total"]], F32)
            nc.sync.dma_start(out=dstv2_sb[:], in_=dstv2_d.ap()[:, :])
            iota_i = cp.tile([128, 512], I16)
            nc.gpsimd.iota(iota_i[:], pattern=[[0, 4], [1, 128]], base=0,
                           channel_multiplier=0)
            iota4 = cp.tile([128, 4, 128], F32)
            nc.vector.tensor_copy(out=iota4[:],
                                  in_=iota_i[:].rearrange("p (c f) -> p c f",
                                                          c=4))

            # ---- phase A: z1 = dinvA * (x @ W1) -> ag1_in
            with (
                tc.tile_pool(name="xin", bufs=4) as xp,
                tc.tile_pool(name="xt", bufs=4) as xtp,
                tc.tile_pool(name="zs", bufs=4) as zp,
                tc.tile_pool(name="psA", bufs=4, space="PSUM") as psA,
                tc.tile_pool(name="psAT", bufs=4, space="PSUM") as psAT,
            ):
                for t in range(TPC):
                    xt_in = xp.tile([128, IN_CH], F32)
                    nc.sync.dma_start(out=xt_in[:],
                                      in_=x_d.ap()[t * 128:(t + 1) * 128, :])
                    xT_ps = psAT.tile([IN_CH, 128], F32, space="PSUM")
                    nc.tensor.transpose(out=xT_ps[:], in_=xt_in[:],
                                        identity=ident[:])
                    xT = xtp.tile([IN_CH, 128], F32)
                    nc.vector.tensor_copy(out=xT[:], in_=xT_ps[:])
                    zps = psA.tile([128, HID], F32, space="PSUM")
                    nc.tensor.matmul(out=zps[:], lhsT=xT[:], rhs=w1sb[:],
                                     start=True, stop=True)
                    zsb = zp.tile([128, HID], F32)
                    nc.vector.tensor_scalar(out=zsb[:], in0=zps[:],
                                            scalar1=dinvA_sb[:, t:t + 1],
                                            scalar2=None,
                                            op0=mybir.AluOpType.mult)
                    nc.sync.dma_start(out=ag1_in[t * 128:(t + 1) * 128, :],
                                      in_=zsb[:])

            nc.gpsimd.collective_compute(
                "AllGather", mybir.AluOpType.bypass,
                replica_groups=[list(range(N_CORES))],
                ins=[ag1_in[:]], outs=[ag1_out[:]])

            _emit_agg(nc, tc, cp, ag1_out, idx1_sb, dstv1_sb, dinvS_sb,
                      bb1_sb, iota4, L1, layer=1, out_d=ag2_in)

            nc.gpsimd.collective_compute(
                "AllGather", mybir.AluOpType.bypass,
                replica_groups=[list(range(N_CORES))],
                ins=[ag2_in[:]], outs=[ag2_out[:]])

            _emit_agg(nc, tc, cp, ag2_out, idx2_sb, dstv2_sb, dinvS_sb,
                      bb2_sb, iota4, L2, layer=2, out_d=out_d.ap(),
                      w2sb=w2sb, ident=ident)
    nc.compile()
    return nc


class _SpmdRunner:
    def __init__(self, nc, n_cores=N_CORES):
        bass2jax.install_neuronx_cc_hook()
        self.nc = nc
        self.n_cores = n_cores
        in_names, out_names, out_avals = [], [], []
        partition_name = nc.partition_id_tensor.name if nc.partition_id_tensor \
            else None
        for alloc in nc.m.functions[0].allocations:
            if not isinstance(alloc, mybir.MemoryLocationSet):
                continue
            name = alloc.memorylocations[0].name
            if alloc.kind == "ExternalInput":
                if name != partition_name:
                    in_names.append(name)
            elif alloc.kind == "ExternalOutput":
                out_names.append(name)
                out_avals.append(jax.core.ShapedArray(
                    tuple(alloc.tensor_shape), mybir.dt.np(alloc.dtype)))
        self.in_names, self.out_names, self.out_avals = \
            in_names, out_names, out_avals
        n_params = len(in_names)
        n_outs = len(out_avals)
        all_names = list(in_names) + list(out_names)
        if partition_name is not None:
            all_names.append(partition_name)

        def _body(*args):
            operands = list(args)
            if partition_name is not None:
                operands.append(partition_id_tensor())
            outs = _bass_exec_p.bind(
                *operands,
                out_avals=tuple(out_avals),
                in_names=tuple(all_names),
                out_names=tuple(out_names),
                lowering_input_output_aliases=(),
                sim_require_finite=True,
                sim_require_nnan=True,
                nc=nc,
            )
            return tuple(outs)

        devices = jax.devices()[:n_cores]
        assert len(devices) >= n_cores or len(devices) == n_cores, \
            f"need {n_cores} cores, have {len(jax.devices())}"
        self.mesh = Mesh(np.asarray(devices), ("core",))
        in_specs = (PartitionSpec("core"),) * (n_params + n_outs)
        out_specs = (PartitionSpec("core"),) * n_outs
        self.fn = jax.jit(
            shard_map(_body, mesh=self.mesh, in_specs=in_specs,
                      out_specs=out_specs, check_rep=False),
            keep_unused=True,
        )

    def run(self, in_maps):
        concat_in = [
            np.concatenate([np.asarray(in_maps[c][nm])
                            for c in range(self.n_cores)], axis=0)
            for nm in self.in_names
        ]
        concat_zeros = [
            np.zeros((self.n_cores * av.shape[0], *av.shape[1:]), av.dtype)
            for av in self.out_avals
        ]
        outs = self.fn(*(concat_in + concat_zeros))
        jax.block_until_ready(outs)
        res = []
        for c in range(self.n_cores):
            d = {}
            for i, nm in enumerate(self.out_names):
                a = np.asarray(outs[i]).reshape(self.n_cores,
                                                *self.out_avals[i].shape)
                d[nm] = a[c]
            res.append(d)
        return res


_CACHE = {}


def _get_programs(edge_index):
    key = hash(np.asarray(edge_index).tobytes())
    if key not in _CACHE:
        pre = _preprocess(edge_index)
        ncM = _build_merged(pre)
        _CACHE[key] = (pre, _SpmdRunner(ncM))
    return _CACHE[key]


def _make_maps(pre, x, W1, b1, W2, b2):
    dinv = pre["dinv"]
    bb1 = np.tile(b1, (128, 1)).astype(np.float32)
    bb2 = np.tile(b2, (128, 1)).astype(np.float32)
    maps = []
    for c in range(N_CORES):
        lo = c * ROWS
        hi = min((c + 1) * ROWS, N_NODES)
        xs = np.zeros((ROWS, IN_CH), np.float32)
        xs[:hi - lo] = x[lo:hi]
        maps.append({
            "x": xs, "w1": W1, "w2": W2,
            "dinvA": pre["dinvA"][c], "dinvS": pre["dinvS"][c],
            "bb1": bb1, "bb2": bb2,
            "idxw1": pre["L1"]["idxw"][c], "dstv1": pre["L1"]["dstv"][c],
            "idxw2": pre["L2"]["idxw"][c], "dstv2": pre["L2"]["dstv"][c],
        })
    return maps


def kernel(x, edge_index, W1, b1, W2, b2):
    x = np.asarray(x, np.float32)
    W1 = np.asarray(W1, np.float32)
    b1 = np.asarray(b1, np.float32)
    W2 = np.asarray(W2, np.float32)
    b2 = np.asarray(b2, np.float32)
    pre, rM = _get_programs(edge_index)
    maps = _make_maps(pre, x, W1, b1, W2, b2)
    res = rM.run(maps)
    tile_of = pre["tile_of"]
    out = np.zeros((N_NODES, OUT_CH), np.float32)
    for c in range(N_CORES):
        o = res[c]["outp"]
        for t in range(TPC):
            tl = tile_of[c, t]
            if tl >= 0:
                out[tl * 128:(tl + 1) * 128] = o[t * 128:(t + 1) * 128]
    return out
